# revision 1
# baseline (speedup 1.0000x reference)
"""MultiHeadedAttention on 8 Trainium2 NeuronCores.

Sharding: sequence-sharded. Cores 0-3 handle batch 0, cores 4-7 batch 1.
Within a batch group, core c owns query AND key/value tokens [512c, 512c+512).
Each core projects Q/K/V for its own 512 tokens, K^T and augmented-V are
AllGathered within the 4-core group, attention runs over 512 queries x 2048
keys x 16 heads, and the output projection is local (no collective after).

Layouts (all device-side, transposes done on host):
  Q^T, K^T: [feature, token]  (contraction on partitions for matmuls)
  V:        [token, feature] augmented with a ones column per head ->
            PV matmul row 64 yields the softmax denominator for free.
  scores^T: [key, query]; softmax along partitions via the ones column.
  mask applied multiplicatively after exp (mask is 0/1).
"""

import numpy as np

B, S, D, H, DK = 2, 2048, 1024, 16, 64
NCORES = 8
GPB = 4            # cores per batch group
QB = S // GPB      # 512 tokens per core
NI = D // 128      # 8 feature chunks
NKB = S // 128     # 16 key blocks
NVC = H * (DK + 1)  # 1040 augmented V columns

TRACE = False
LAST_EXEC_NS = None
LAST_RESULTS = None
_PROG = None


def _build():
    from concourse import bass, mybir, tile

    FP = mybir.dt.float32
    FR = mybir.dt.float32r
    AF = mybir.ActivationFunctionType
    OP = mybir.AluOpType

    nc = bass.Bass(num_devices=NCORES)

    xq_t = nc.dram_tensor("xq_t", [D, QB], FP, kind="ExternalInput")
    xk_t = nc.dram_tensor("xk_t", [D, QB], FP, kind="ExternalInput")
    xv_t = nc.dram_tensor("xv_t", [D, QB], FP, kind="ExternalInput")
    mask_t = nc.dram_tensor("mask_t", [S, QB], FP, kind="ExternalInput")
    wq_t = nc.dram_tensor("wq_t", [D, D], FP, kind="ExternalInput")
    wk_t = nc.dram_tensor("wk_t", [D, D], FP, kind="ExternalInput")
    wv_t = nc.dram_tensor("wv_t", [D, D], FP, kind="ExternalInput")
    wo_t = nc.dram_tensor("wo_t", [D, D], FP, kind="ExternalInput")
    bq2 = nc.dram_tensor("bq2", [128, NI], FP, kind="ExternalInput")
    bk2 = nc.dram_tensor("bk2", [128, NI], FP, kind="ExternalInput")
    bv_b = nc.dram_tensor("bv_b", [128, D], FP, kind="ExternalInput")
    bo_b = nc.dram_tensor("bo_b", [128, D], FP, kind="ExternalInput")
    out = nc.dram_tensor("out", [QB, D], FP, kind="ExternalOutput")

    groups = [[0, 1, 2, 3], [4, 5, 6, 7]]

    with tile.TileContext(nc) as tc:
        with tc.tile_pool(name="dram", bufs=1, space="DRAM") as dpool, \
             tc.tile_pool(name="persist", bufs=1) as pp:

            kt_in = dpool.tile([D, QB], FP, tag="kt_in")
            v_in = dpool.tile([QB, NVC], FP, tag="v_in")
            kt_g = dpool.tile([GPB * D, QB], FP, tag="kt_g")
            v_g = dpool.tile([GPB * QB, NVC], FP, tag="v_g")

            qt = pp.tile([128, NI, QB], FR, tag="qt")
            at = pp.tile([128, NI, QB], FR, tag="at")
            msk = pp.tile([128, NKB, QB], FR, tag="msk")
            bq_sb = pp.tile([128, NI], FP, tag="bq")
            bk_sb = pp.tile([128, NI], FP, tag="bk")
            bv_bc = pp.tile([128, D], FP, tag="bv")
            bo_bc = pp.tile([128, D], FP, tag="bo")
            ones_c = pp.tile([1, 64], FP, tag="ones_c")

            # bias prep (bv_b/bo_b arrive pre-broadcast from host)
            nc.sync.dma_start(bq_sb[:], bq2[:])
            nc.sync.dma_start(bk_sb[:], bk2[:])
            nc.sync.dma_start(bv_bc[:], bv_b[:])
            nc.sync.dma_start(bo_bc[:], bo_b[:])
            nc.gpsimd.memset(ones_c[:], 1.0)

            # mask load
            for kb in range(NKB):
                nc.sync.dma_start(msk[:, kb, :],
                                  mask_t[kb * 128:(kb + 1) * 128, :].bitcast(FR))

            # ---------------- projection phase ----------------
            with tc.tile_pool(name="wstage", bufs=2) as wp, \
                 tc.tile_pool(name="xstage", bufs=2) as xp, \
                 tc.tile_pool(name="kst", bufs=2) as kstp, \
                 tc.tile_pool(name="vst", bufs=2) as vstp, \
                 tc.tile_pool(name="psA", bufs=3, space="PSUM") as psA:

                def load_x(src):
                    x_sb = xp.tile([128, NI, QB], FR, name="x_sb")
                    for ci in range(NI):
                        nc.sync.dma_start(x_sb[:, ci, :],
                                          src[ci * 128:(ci + 1) * 128, :].bitcast(FR))
                    return x_sb

                def load_w_half(src, oh):
                    w_sb = wp.tile([128, NI, 512], FR, name="w_sb")
                    for ci in range(NI):
                        nc.sync.dma_start(
                            w_sb[:, ci, :],
                            src[ci * 128:(ci + 1) * 128,
                                oh * 512:(oh + 1) * 512].bitcast(FR))
                    return w_sb

                # K projection -> kt_in DRAM (feature-major), then AllGather
                xk_sb = load_x(xk_t)
                for oh in range(2):
                    wk_sb = load_w_half(wk_t, oh)
                    for obl in range(4):
                        ob = oh * 4 + obl
                        ps = psA.tile([128, QB], FP, name="ps")
                        for ci in range(NI):
                            nc.tensor.matmul(
                                ps[:],
                                lhsT=wk_sb[:, ci, obl * 128:(obl + 1) * 128],
                                rhs=xk_sb[:, ci, :],
                                start=(ci == 0), stop=(ci == NI - 1))
                        kstage = kstp.tile([128, QB], FP, name="kstage")
                        ps_bc, bk_bc = bass.broadcast_tensor_aps(
                            ps[:], bk_sb[:, ob:ob + 1])
                        nc.vector.tensor_tensor(kstage[:], ps_bc, bk_bc,
                                                mybir.AluOpType.add)
                        nc.sync.dma_start(kt_in[ob * 128:(ob + 1) * 128, :],
                                          kstage[:])
                nc.gpsimd.collective_compute(
                    "AllGather", mybir.AluOpType.bypass,
                    replica_groups=groups, ins=[kt_in[:]], outs=[kt_g[:]])

                # V projection -> v_in DRAM (token-major, ones-augmented)
                xv_sb = load_x(xv_t)
                wv_sbs = [load_w_half(wv_t, oh) for oh in range(2)]
                for tb in range(4):
                    vstage = vstp.tile([128, H, DK + 1], FP, name="vstage")
                    nc.gpsimd.memset(vstage[:, :, DK:DK + 1], 1.0)
                    for oh in range(2):
                        ps = psA.tile([128, 512], FP, name="ps")
                        for ci in range(NI):
                            nc.tensor.matmul(
                                ps[:],
                                lhsT=xv_sb[:, ci, tb * 128:(tb + 1) * 128],
                                rhs=wv_sbs[oh][:, ci, :],
                                start=(ci == 0), stop=(ci == NI - 1))
                        for hh in range(8):
                            h = oh * 8 + hh
                            nc.vector.tensor_tensor(
                                vstage[:, h, 0:DK],
                                ps[:, hh * DK:(hh + 1) * DK],
                                bv_bc[:, h * DK:(h + 1) * DK],
                                mybir.AluOpType.add)
                    nc.sync.dma_start(v_in[tb * 128:(tb + 1) * 128, :],
                                      vstage[:].opt())
                nc.gpsimd.collective_compute(
                    "AllGather", mybir.AluOpType.bypass,
                    replica_groups=groups, ins=[v_in[:]], outs=[v_g[:]])

                # Q projection -> qt SBUF (feature-major), stays local
                xq_sb = load_x(xq_t)
                for oh in range(2):
                    wq_sb = load_w_half(wq_t, oh)
                    for obl in range(4):
                        ob = oh * 4 + obl
                        ps = psA.tile([128, QB], FP, name="ps")
                        for ci in range(NI):
                            nc.tensor.matmul(
                                ps[:],
                                lhsT=wq_sb[:, ci, obl * 128:(obl + 1) * 128],
                                rhs=xq_sb[:, ci, :],
                                start=(ci == 0), stop=(ci == NI - 1))
                        ps_bc, bq_bc = bass.broadcast_tensor_aps(
                            ps[:], bq_sb[:, ob:ob + 1])
                        nc.vector.tensor_tensor(qt[:, ob, :], ps_bc, bq_bc,
                                                mybir.AluOpType.add)

            # ---------------- attention + output phase ----------------
            with tc.tile_pool(name="wo", bufs=1) as wop:
                vaug = wop.tile([128, NKB, H, DK + 1], FR, tag="vaug")
                wo_sb = wop.tile([128, NI, D], FR, tag="wo_sb")
                for ci in range(NI):
                    nc.scalar.dma_start(wo_sb[:, ci, :],
                                        wo_t[ci * 128:(ci + 1) * 128, :].bitcast(FR))

                # V gather readback into SBUF (all ranks, incl. own)
                for kc in range(NKB):
                    nc.sync.dma_start(vaug[:, kc, :, :].opt(),
                                      v_g[kc * 128:(kc + 1) * 128, :].bitcast(FR))

                with tc.tile_pool(name="kstream", bufs=4) as kp, \
                     tc.tile_pool(name="probs", bufs=3) as pbp, \
                     tc.tile_pool(name="rb", bufs=2) as rbp, \
                     tc.tile_pool(name="scps", bufs=2, space="PSUM") as scp, \
                     tc.tile_pool(name="rbps", bufs=2, space="PSUM") as rbps, \
                     tc.tile_pool(name="pvps", bufs=2, space="PSUM") as pvp:

                    for h in range(H):
                        ci_h = h // 2
                        p0 = 64 * (h % 2)
                        q_ap = qt[p0:p0 + 64, ci_h, :]
                        pv = pvp.tile([DK + 1, QB], FP, name="pv")
                        for kb2 in range(NKB // 2):
                            rk = kb2 // 2
                            l0 = (kb2 % 2) * 256
                            kstr = kp.tile([128, 256], FR, name="kstr")
                            row = rk * D + ci_h * 128 + p0
                            nc.sync.dma_start(kstr[p0:p0 + 64, :],
                                              kt_g[row:row + 64,
                                                   l0:l0 + 256].bitcast(FR))
                            sc = scp.tile([128, 2, 512], FP, name="sc")
                            for j in range(2):
                                nc.tensor.matmul(
                                    sc[:, j, :],
                                    lhsT=kstr[p0:p0 + 64,
                                              j * 128:(j + 1) * 128],
                                    rhs=q_ap,
                                    start=True, stop=True)
                            pb = pbp.tile([128, 2, 512], FR, name="pb")
                            nc.scalar.activation(pb[:], sc[:], AF.Exp, scale=0.125)
                            nc.vector.tensor_tensor(
                                pb[:], pb[:], msk[:, 2 * kb2:2 * kb2 + 2, :],
                                mybir.AluOpType.mult)
                            for j in range(2):
                                kb = kb2 * 2 + j
                                nc.tensor.matmul(
                                    pv[:],
                                    lhsT=vaug[:, kb, h, :],
                                    rhs=pb[:, j, :],
                                    start=(kb == 0), stop=(kb == NKB - 1))
                        nc.vector.tensor_copy(at[p0:p0 + 64, ci_h, :], pv[0:DK, :])
                        rden = rbp.tile([1, QB], FP, name="rden")
                        nc.vector.reciprocal(rden[0:1, :], pv[DK:DK + 1, :])
                        # broadcast 1/denom across 64 partitions via K=1 matmul
                        rbq = rbps.tile([128, QB], FP, name="rbq")
                        nc.tensor.matmul(rbq[p0:p0 + 64, :],
                                         lhsT=ones_c[0:1, :],
                                         rhs=rden[0:1, :],
                                         start=True, stop=True)
                        nc.vector.tensor_tensor(
                            at[p0:p0 + 64, ci_h, :], at[p0:p0 + 64, ci_h, :],
                            rbq[p0:p0 + 64, :], mybir.AluOpType.mult)

                # output projection: out[q, o] with A^T stationary
                with tc.tile_pool(name="outsb", bufs=2) as osp, \
                     tc.tile_pool(name="pops", bufs=2, space="PSUM") as pop:
                    for qb in range(4):
                        osb = osp.tile([128, D], FP, name="osb")
                        for oh in range(2):
                            po = pop.tile([128, 512], FP, name="po")
                            for ci in range(NI):
                                nc.tensor.matmul(
                                    po[:],
                                    lhsT=at[:, ci, qb * 128:(qb + 1) * 128],
                                    rhs=wo_sb[:, ci, oh * 512:(oh + 1) * 512],
                                    start=(ci == 0), stop=(ci == NI - 1))
                            nc.vector.tensor_tensor(
                                osb[:, oh * 512:(oh + 1) * 512], po[:],
                                bo_bc[:, oh * 512:(oh + 1) * 512],
                                mybir.AluOpType.add)
                        nc.sync.dma_start(out[qb * 128:(qb + 1) * 128, :], osb[:])
    return nc


_PATCHED = False


def _split_multi_waits(bir_bytes):
    # This walrus build allows only one sync-wait command per instruction.
    # Hoist extra waits onto EventSemaphore carriers just before each
    # instruction in the same engine stream (engines execute in order).
    import json
    j = json.loads(bir_bytes)
    for fn in j["functions"]:
        for blk in fn["blocks"]:
            out = []
            for inst in blk["instructions"]:
                si = inst.get("sync_info") or {}
                waits = si.get("on_wait") or []
                if len(waits) > 1:
                    for k, w in enumerate(waits[:-1]):
                        out.append({
                            "debug": inst.get("debug", 0),
                            "engine": inst["engine"],
                            "ins": [],
                            "name": f"{inst['name']}_w{k}",
                            "opcode": "EventSemaphore",
                            "outs": [],
                            "sync_info": {"on_update": [], "on_wait": [w]},
                        })
                    si["on_wait"] = [waits[-1]]
                out.append(inst)
            blk["instructions"] = out
    return json.dumps(j).encode()


def _patch_compiler():
    global _PATCHED
    if _PATCHED:
        return
    from concourse import bass_utils, bass2jax
    orig = bass_utils.compile_bir_kernel

    def wrapped(bir_json, tmpdir, neff_name="file.neff"):
        return orig(_split_multi_waits(bir_json), tmpdir, neff_name)

    bass_utils.compile_bir_kernel = wrapped
    bass2jax.compile_bir_kernel = wrapped
    _PATCHED = True


def kernel(query, key, value, mask, Wq, bq, Wk, bk, Wv, bv, Wo, bo):
    global LAST_EXEC_NS, LAST_RESULTS, _PROG
    _patch_compiler()
    from concourse.bass_utils import run_bass_kernel_spmd

    f32 = np.float32
    wq_t = np.ascontiguousarray(np.asarray(Wq, dtype=f32).T)
    wk_t = np.ascontiguousarray(np.asarray(Wk, dtype=f32).T)
    wv_t = np.ascontiguousarray(np.asarray(Wv, dtype=f32).T)
    wo_t = np.ascontiguousarray(np.asarray(Wo, dtype=f32).T)
    bq2 = np.ascontiguousarray(np.asarray(bq, dtype=f32).reshape(NI, 128).T)
    bk2 = np.ascontiguousarray(np.asarray(bk, dtype=f32).reshape(NI, 128).T)
    bv_b = np.ascontiguousarray(
        np.broadcast_to(np.asarray(bv, dtype=f32).reshape(1, D), (128, D)))
    bo_b = np.ascontiguousarray(
        np.broadcast_to(np.asarray(bo, dtype=f32).reshape(1, D), (128, D)))

    in_maps = []
    for r in range(NCORES):
        b, c = divmod(r, GPB)
        q0 = QB * c
        in_maps.append({
            "xq_t": np.ascontiguousarray(
                np.asarray(query[b, q0:q0 + QB, :], dtype=f32).T),
            "xk_t": np.ascontiguousarray(
                np.asarray(key[b, q0:q0 + QB, :], dtype=f32).T),
            "xv_t": np.ascontiguousarray(
                np.asarray(value[b, q0:q0 + QB, :], dtype=f32).T),
            "mask_t": np.ascontiguousarray(
                np.asarray(mask[b, q0:q0 + QB, :], dtype=f32).T),
            "wq_t": wq_t, "wk_t": wk_t, "wv_t": wv_t, "wo_t": wo_t,
            "bq2": bq2, "bk2": bk2, "bv_b": bv_b, "bo_b": bo_b,
        })

    if _PROG is None:
        _PROG = _build()

    res = run_bass_kernel_spmd(_PROG, in_maps, core_ids=list(range(NCORES)),
                               trace=TRACE)
    LAST_EXEC_NS = res.exec_time_ns
    LAST_RESULTS = res

    out_full = np.empty((B, S, D), dtype=f32)
    for r in range(NCORES):
        b, c = divmod(r, GPB)
        q0 = QB * c
        out_full[b, q0:q0 + QB, :] = res.results[r]["out"]
    return out_full



# revision 18
# speedup vs baseline: 1.8751x; 1.8751x over previous
"""MultiHeadedAttention on 8 Trainium2 NeuronCores.

Sharding: sequence-sharded queries, replicated keys. Cores 0-3 handle
batch 0, cores 4-7 batch 1. Within a batch group, core c owns query
tokens [512c, 512c+512) but receives the FULL key/value inputs and
computes the complete K and V projections locally — no collectives,
no inter-core barrier, no AllGather on the critical path.

Everything is bf16 except PSUM accumulation, softmax denominators and
the final output (fp32). bf16 matmuls run at 1 cycle/row (vs ~1.5 for
fp32r) and enable the DVE 2x packed mode for the mask multiply.

Layouts (host-side transposes):
  kt:   [feature, key]    SBUF-resident, produced by local K projection
  qt:   [feature, query]  scores contract on the dk partitions
  vaug: [key, h, dk+1]    ones column -> PV row 64 = softmax denominator
  scores^T: [key, query]; mask applied multiplicatively after exp.

Tensor-engine emission order keeps the engine busy in-order: V proj
(full batch), Q proj, K proj feature-block 0, then attention heads with
remaining K-proj feature blocks interleaved as filler while the scalar
engine's exp stream catches up.
"""

import numpy as np

B, S, D, H, DK = 2, 2048, 1024, 16, 64
NCORES = 8
GPB = 4             # cores per batch group
QB = S // GPB       # 512 query tokens per core
NI = D // 128       # 8 feature chunks
NKB = S // 128      # 16 key blocks

TRACE = False
LAST_EXEC_NS = None
LAST_RESULTS = None
_PROG = None


def _build():
    from concourse import bass, mybir, tile

    FP = mybir.dt.float32
    BF = mybir.dt.bfloat16
    AF = mybir.ActivationFunctionType
    OP = mybir.AluOpType

    nc = bass.Bass(num_devices=NCORES)

    xq_t = nc.dram_tensor("xq_t", [D, QB], BF, kind="ExternalInput")
    xk_t = nc.dram_tensor("xk_t", [D, S], BF, kind="ExternalInput")
    xv_t = nc.dram_tensor("xv_t", [D, S], BF, kind="ExternalInput")
    mask_t = nc.dram_tensor("mask_t", [S, QB], BF, kind="ExternalInput")
    wq_t = nc.dram_tensor("wq_t", [D, D], BF, kind="ExternalInput")
    wk_t = nc.dram_tensor("wk_t", [D, D], BF, kind="ExternalInput")
    wv_t = nc.dram_tensor("wv_t", [D, D], BF, kind="ExternalInput")
    wo_t = nc.dram_tensor("wo_t", [D, D], BF, kind="ExternalInput")
    bq2 = nc.dram_tensor("bq2", [128, NI], FP, kind="ExternalInput")
    bk2 = nc.dram_tensor("bk2", [128, NI], FP, kind="ExternalInput")
    bv_b = nc.dram_tensor("bv_b", [128, D], FP, kind="ExternalInput")
    bo_b = nc.dram_tensor("bo_b", [128, D], FP, kind="ExternalInput")
    sel_in = nc.dram_tensor("sel_in", [H, NI * 128], BF, kind="ExternalInput")
    out = nc.dram_tensor("out", [QB, D], FP, kind="ExternalOutput")

    with tile.TileContext(nc) as tc:
        with tc.tile_pool(name="dram", bufs=1, space="DRAM") as dpool, \
             tc.tile_pool(name="persist", bufs=1) as pp:
            dens_d = dpool.tile([H, QB], FP, tag="dens_d")
            kt = pp.tile([128, NI, S], BF, tag="kt")
            vaug = pp.tile([128, NKB, H, DK + 1], BF, tag="vaug")
            qt = pp.tile([128, NI, QB], BF, tag="qt")
            msk = pp.tile([128, NKB, QB], BF, tag="msk")
            at = pp.tile([128, NI, QB], BF, tag="at")
            wo_sb = pp.tile([128, NI, D], BF, tag="wo_sb")
            xk_sb = pp.tile([128, NI, S], BF, tag="xk_sb")
            densP = pp.tile([H, QB], FP, tag="densP")
            rdens = pp.tile([H, QB], BF, tag="rdens")
            bq_sb = pp.tile([128, NI], FP, tag="bq")
            bk_sb = pp.tile([128, NI], FP, tag="bk")
            bv_bc = pp.tile([128, D], FP, tag="bv")
            bo_bc = pp.tile([128, D], FP, tag="bo")
            # selector for denom broadcast: sel[k, ci, p] = 1 iff head
            # k == 2*ci + p//64; sel_ci^T @ rdens -> [128, 512] per-chunk
            # reciprocal denominators for both heads of chunk ci.
            sel = pp.tile([H, NI, 128], BF, tag="sel")
            nc.sync.dma_start(sel[:], sel_in[:])

            nc.sync.dma_start(bq_sb[:], bq2[:])
            nc.sync.dma_start(bk_sb[:], bk2[:])
            nc.sync.dma_start(bv_bc[:], bv_b[:])
            nc.sync.dma_start(bo_bc[:], bo_b[:])
            nc.gpsimd.memset(vaug[:], 1.0)  # ones column; V cols overwritten

            for kb in range(NKB):
                nc.sync.dma_start(msk[:, kb, :],
                                  mask_t[kb * 128:(kb + 1) * 128, :])
            for ci in range(NI):
                nc.scalar.dma_start(wo_sb[:, ci, :],
                                    wo_t[ci * 128:(ci + 1) * 128, :])
                nc.sync.dma_start(xk_sb[:, ci, :],
                                  xk_t[ci * 128:(ci + 1) * 128, :])

            with tc.tile_pool(name="psA", bufs=2, space="PSUM") as psA:
                # ------------- V projection (full batch) -------------
                # out[tok, vf] = xv_chunk^T @ wv ; vaug[tok, h, 0:64] strided.
                with tc.tile_pool(name="vw", bufs=1) as vwp, \
                     tc.tile_pool(name="vx", bufs=2) as vxp:
                    wv_sb = vwp.tile([128, NI, D], BF, name="wv_sb")
                    for ci in range(NI):
                        nc.sync.dma_start(wv_sb[:, ci, :],
                                          wv_t[ci * 128:(ci + 1) * 128, :])
                    for tg in range(4):  # 4 groups of 512 tokens
                        xv_sb = vxp.tile([128, NI, 512], BF, name="xv_sb")
                        for ci in range(NI):
                            nc.sync.dma_start(
                                xv_sb[:, ci, :],
                                xv_t[ci * 128:(ci + 1) * 128,
                                     tg * 512:(tg + 1) * 512])
                        for tbl in range(4):
                            tb = tg * 4 + tbl
                            for oh in range(2):
                                ps = psA.tile([128, 512], FP, name="ps")
                                for ci in range(NI):
                                    nc.tensor.matmul(
                                        ps[:],
                                        lhsT=xv_sb[:, ci,
                                                   tbl * 128:(tbl + 1) * 128],
                                        rhs=wv_sb[:, ci,
                                                  oh * 512:(oh + 1) * 512],
                                        start=(ci == 0), stop=(ci == NI - 1))
                                nc.vector.tensor_tensor(
                                    vaug[:, tb, oh * 8:(oh + 1) * 8, 0:DK],
                                    ps[:].rearrange("p (h d) -> p h d", h=8),
                                    bv_bc[:, oh * 512:(oh + 1) * 512]
                                        .rearrange("p (h d) -> p h d", h=8),
                                    OP.add)

                # ---------------- Q projection ----------------
                with tc.tile_pool(name="qstage", bufs=1) as qsp:
                    wq_sb = qsp.tile([128, NI, D], BF, name="wq_sb")
                    xq_sb = qsp.tile([128, NI, QB], BF, name="xq_sb")
                    for ci in range(NI):
                        nc.sync.dma_start(wq_sb[:, ci, :],
                                          wq_t[ci * 128:(ci + 1) * 128, :])
                        nc.sync.dma_start(xq_sb[:, ci, :],
                                          xq_t[ci * 128:(ci + 1) * 128, :])
                    for ob in range(NI):
                        ps = psA.tile([128, QB], FP, name="ps")
                        for ci in range(NI):
                            nc.tensor.matmul(
                                ps[:],
                                lhsT=wq_sb[:, ci, ob * 128:(ob + 1) * 128],
                                rhs=xq_sb[:, ci, :],
                                start=(ci == 0), stop=(ci == NI - 1))
                        nc.scalar.activation(qt[:, ob, :], ps[:], AF.Identity,
                                             bias=bq_sb[:, ob:ob + 1])

                # -------- K projection + attention, interleaved --------
                wk_sb = pp.tile([128, NI, D], BF, tag="wk_sb")
                for ci in range(NI):
                    nc.sync.dma_start(wk_sb[:, ci, :],
                                      wk_t[ci * 128:(ci + 1) * 128, :])

                k_groups = [(fb, tc4) for fb in range(NI) for tc4 in range(4)]

                def emit_k_group():
                    # one [128, fb-block] x [512 keys] accumulation group of
                    # the K projection; interleaved into the attention loop
                    # as tensor-engine filler.
                    if not k_groups:
                        return
                    fb, tc4 = k_groups.pop(0)
                    ps = psA.tile([128, 512], FP, name="ps")
                    for ci in range(NI):
                        nc.tensor.matmul(
                            ps[:],
                            lhsT=wk_sb[:, ci, fb * 128:(fb + 1) * 128],
                            rhs=xk_sb[:, ci, tc4 * 512:(tc4 + 1) * 512],
                            start=(ci == 0), stop=(ci == NI - 1))
                    ps_bc, bk_bc = bass.broadcast_tensor_aps(
                        ps[:], bk_sb[:, fb:fb + 1])
                    nc.vector.tensor_tensor(
                        kt[:, fb, tc4 * 512:(tc4 + 1) * 512],
                        ps_bc, bk_bc, OP.add)

                with tc.tile_pool(name="probs", bufs=6) as pbp, \
                     tc.tile_pool(name="dscr", bufs=3) as dsp, \
                     tc.tile_pool(name="scps", bufs=2, space="PSUM") as scp, \
                     tc.tile_pool(name="pvps", bufs=2, space="PSUM") as pvp:

                    def attn_head(h):
                        # software-pipelined emission: scores for chunk k+2
                        # are emitted before PV of chunk k so the in-order
                        # tensor engine never waits on the exp+mask latency.
                        ci_h = h // 2
                        p0 = 64 * (h % 2)
                        q_ap = qt[p0:p0 + 64, ci_h, :]
                        pv = pvp.tile([DK + 1, QB], FP, name="pv")
                        pbs = {}

                        def emit_pv(kb2):
                            pb = pbs.pop(kb2)
                            for j in range(2):
                                kb = kb2 * 2 + j
                                nc.tensor.matmul(
                                    pv[:],
                                    lhsT=vaug[:, kb, h, :],
                                    rhs=pb[:, j, :],
                                    start=(kb == 0), stop=(kb == NKB - 1))

                        for kb2 in range(NKB // 2):
                            sc = scp.tile([128, 2, 512], FP, name="sc")
                            for j in range(2):
                                kb = kb2 * 2 + j
                                nc.tensor.matmul(
                                    sc[:, j, :],
                                    lhsT=kt[p0:p0 + 64, ci_h,
                                            kb * 128:(kb + 1) * 128],
                                    rhs=q_ap,
                                    start=True, stop=True)
                            pb = pbp.tile([128, 2, 512], BF, name="pb")
                            nc.scalar.activation(pb[:], sc[:], AF.Exp,
                                                 scale=0.125)
                            nc.vector.tensor_tensor(
                                pb[:], pb[:],
                                msk[:, 2 * kb2:2 * kb2 + 2, :], OP.mult)
                            pbs[kb2] = pb
                            if kb2 in (1, 5):
                                emit_k_group()  # K-proj filler, 2 per head
                            if kb2 >= 2:
                                emit_pv(kb2 - 2)
                        emit_pv(NKB // 2 - 2)
                        emit_pv(NKB // 2 - 1)
                        nc.vector.tensor_copy(at[p0:p0 + 64, ci_h, :],
                                              pv[0:DK, :])
                        dscr = dsp.tile([1, QB], FP, name="dscr")
                        nc.vector.tensor_copy(dscr[:], pv[DK:DK + 1, :])
                        nc.sync.dma_start(dens_d[h:h + 1, :], dscr[:])

                    for _ in range(4):
                        emit_k_group()  # fb0 upfront for head 0
                    for h in range(H):
                        attn_head(h)

                    # normalize: at[:, ci, :] *= 1/denom, broadcast across
                    # partitions via a selector matmul (2 heads per tile)
                    nc.sync.dma_start(densP[:], dens_d[:])
                    with nc.allow_low_precision(
                            reason="1/denom in bf16; denom O(1e3), rel "
                                   "err ~0.4% is within tolerance"):
                        nc.vector.reciprocal(rdens[:], densP[:])
                    for ci in range(NI):
                        rbq = pvp.tile([128, QB], FP, name="pv")
                        nc.tensor.matmul(rbq[:], lhsT=sel[:, ci, :],
                                         rhs=rdens[:], start=True, stop=True)
                        nc.vector.tensor_tensor(at[:, ci, :], at[:, ci, :],
                                                rbq[:], OP.mult)

                # ---------------- output projection ----------------
                with tc.tile_pool(name="outsb", bufs=2) as osp:
                    for qb in range(4):
                        osb = osp.tile([128, D], FP, name="osb")
                        for oh in range(2):
                            po = psA.tile([128, 512], FP, name="ps")
                            for ci in range(NI):
                                nc.tensor.matmul(
                                    po[:],
                                    lhsT=at[:, ci, qb * 128:(qb + 1) * 128],
                                    rhs=wo_sb[:, ci, oh * 512:(oh + 1) * 512],
                                    start=(ci == 0), stop=(ci == NI - 1))
                            nc.vector.tensor_tensor(
                                osb[:, oh * 512:(oh + 1) * 512], po[:],
                                bo_bc[:, oh * 512:(oh + 1) * 512], OP.add)
                        nc.sync.dma_start(out[qb * 128:(qb + 1) * 128, :],
                                          osb[:])
    return nc


_PATCHED = False


def _split_multi_waits(bir_bytes):
    # This walrus build allows only one sync-wait command per instruction.
    # Hoist extra waits onto EventSemaphore carriers just before each
    # instruction in the same engine stream (engines execute in order).
    import json
    j = json.loads(bir_bytes)
    for fn in j["functions"]:
        for blk in fn["blocks"]:
            out = []
            for inst in blk["instructions"]:
                si = inst.get("sync_info") or {}
                waits = si.get("on_wait") or []
                if len(waits) > 1:
                    for k, w in enumerate(waits[:-1]):
                        out.append({
                            "debug": inst.get("debug", 0),
                            "engine": inst["engine"],
                            "ins": [],
                            "name": f"{inst['name']}_w{k}",
                            "opcode": "EventSemaphore",
                            "outs": [],
                            "sync_info": {"on_update": [], "on_wait": [w]},
                        })
                    si["on_wait"] = [waits[-1]]
                out.append(inst)
            blk["instructions"] = out
    return json.dumps(j).encode()


def _patch_compiler():
    global _PATCHED
    if _PATCHED:
        return
    from concourse import bass_utils, bass2jax
    orig = bass_utils.compile_bir_kernel

    def wrapped(bir_json, tmpdir, neff_name="file.neff"):
        return orig(_split_multi_waits(bir_json), tmpdir, neff_name)

    bass_utils.compile_bir_kernel = wrapped
    bass2jax.compile_bir_kernel = wrapped
    _PATCHED = True


def kernel(query, key, value, mask, Wq, bq, Wk, bk, Wv, bv, Wo, bo):
    global LAST_EXEC_NS, LAST_RESULTS, _PROG
    _patch_compiler()
    import ml_dtypes
    from concourse.bass_utils import run_bass_kernel_spmd

    f32 = np.float32
    bf16 = ml_dtypes.bfloat16

    def tb(a):  # to bf16, C-contiguous
        return np.ascontiguousarray(np.asarray(a, dtype=f32).astype(bf16))

    wq_t = tb(np.asarray(Wq, dtype=f32).T)
    wk_t = tb(np.asarray(Wk, dtype=f32).T)
    wv_t = tb(np.asarray(Wv, dtype=f32).T)
    wo_t = tb(np.asarray(Wo, dtype=f32).T)
    bq2 = np.ascontiguousarray(np.asarray(bq, dtype=f32).reshape(NI, 128).T)
    bk2 = np.ascontiguousarray(np.asarray(bk, dtype=f32).reshape(NI, 128).T)
    bv_b = np.ascontiguousarray(
        np.broadcast_to(np.asarray(bv, dtype=f32).reshape(1, D), (128, D)))
    bo_b = np.ascontiguousarray(
        np.broadcast_to(np.asarray(bo, dtype=f32).reshape(1, D), (128, D)))

    sel_np = np.zeros((H, NI, 128), dtype=f32)
    for ci in range(NI):
        for hh in range(2):
            sel_np[2 * ci + hh, ci, hh * 64:(hh + 1) * 64] = 1.0
    sel_np = np.ascontiguousarray(sel_np.reshape(H, NI * 128).astype(bf16))

    in_maps = []
    for r in range(NCORES):
        b, c = divmod(r, GPB)
        q0 = QB * c
        in_maps.append({
            "xq_t": tb(np.asarray(query[b, q0:q0 + QB, :], dtype=f32).T),
            "xk_t": tb(np.asarray(key[b], dtype=f32).T),
            "xv_t": tb(np.asarray(value[b], dtype=f32).T),
            "mask_t": tb(np.asarray(mask[b, q0:q0 + QB, :], dtype=f32).T),
            "wq_t": wq_t, "wk_t": wk_t, "wv_t": wv_t, "wo_t": wo_t,
            "bq2": bq2, "bk2": bk2, "bv_b": bv_b, "bo_b": bo_b,
            "sel_in": sel_np,
        })

    if _PROG is None:
        _PROG = _build()

    res = run_bass_kernel_spmd(_PROG, in_maps, core_ids=list(range(NCORES)),
                               trace=TRACE)
    LAST_EXEC_NS = res.exec_time_ns
    LAST_RESULTS = res

    out_full = np.empty((B, S, D), dtype=f32)
    for r in range(NCORES):
        b, c = divmod(r, GPB)
        q0 = QB * c
        out_full[b, q0:q0 + QB, :] = res.results[r]["out"]
    return out_full


# revision 22
# speedup vs baseline: 1.9331x; 1.0309x over previous
"""MultiHeadedAttention on 8 Trainium2 NeuronCores.

Sharding: sequence-sharded queries, replicated keys. Cores 0-3 handle
batch 0, cores 4-7 batch 1. Within a batch group, core c owns query
tokens [512c, 512c+512) but receives the FULL key/value inputs and
computes the complete K and V projections locally — no collectives,
no inter-core barrier, no AllGather on the critical path.

Everything is bf16 except PSUM accumulation, softmax denominators and
the final output (fp32). bf16 matmuls run at 1 cycle/row (vs ~1.5 for
fp32r) and enable the DVE 2x packed mode for the mask multiply.

Layouts (host-side transposes):
  kt:   [feature, key]    SBUF-resident, produced by local K projection
  qt:   [feature, query]  scores contract on the dk partitions
  vaug: [key, h, dk+1]    ones column -> PV row 64 = softmax denominator
  scores^T: [key, query]; mask applied multiplicatively after exp.

Tensor-engine emission order keeps the engine busy in-order: V proj
(full batch), Q proj, K proj feature-block 0, then attention heads with
remaining K-proj feature blocks interleaved as filler while the scalar
engine's exp stream catches up.
"""

import numpy as np

B, S, D, H, DK = 2, 2048, 1024, 16, 64
NCORES = 8
GPB = 4             # cores per batch group
QB = S // GPB       # 512 query tokens per core
NI = D // 128       # 8 feature chunks
NKB = S // 128      # 16 key blocks

TRACE = False
LAST_EXEC_NS = None
LAST_RESULTS = None
_PROG = None


def _build():
    from concourse import bass, mybir, tile

    FP = mybir.dt.float32
    BF = mybir.dt.bfloat16
    AF = mybir.ActivationFunctionType
    OP = mybir.AluOpType

    nc = bass.Bass(num_devices=NCORES)

    xq_t = nc.dram_tensor("xq_t", [D, QB], BF, kind="ExternalInput")
    xk_t = nc.dram_tensor("xk_t", [D, S], BF, kind="ExternalInput")
    xv_t = nc.dram_tensor("xv_t", [D, S], BF, kind="ExternalInput")
    mask_t = nc.dram_tensor("mask_t", [S, QB], BF, kind="ExternalInput")
    wq_t = nc.dram_tensor("wq_t", [D, D], BF, kind="ExternalInput")
    wk_t = nc.dram_tensor("wk_t", [D, D], BF, kind="ExternalInput")
    wv_t = nc.dram_tensor("wv_t", [D, D], BF, kind="ExternalInput")
    wo_t = nc.dram_tensor("wo_t", [D, D], BF, kind="ExternalInput")
    bq2 = nc.dram_tensor("bq2", [128, NI], FP, kind="ExternalInput")
    bk2 = nc.dram_tensor("bk2", [128, NI], FP, kind="ExternalInput")
    bv_b = nc.dram_tensor("bv_b", [128, D], BF, kind="ExternalInput")
    bo_b = nc.dram_tensor("bo_b", [128, D], BF, kind="ExternalInput")
    sel_in = nc.dram_tensor("sel_in", [8, NI * 128], BF, kind="ExternalInput")
    out = nc.dram_tensor("out", [QB, D], FP, kind="ExternalOutput")

    with tile.TileContext(nc) as tc:
        with tc.tile_pool(name="dram", bufs=1, space="DRAM") as dpool, \
             tc.tile_pool(name="persist", bufs=1) as pp:
            dens_d = dpool.tile([H, QB], FP, tag="dens_d")
            kt = pp.tile([128, NI, S], BF, tag="kt")
            vaug = pp.tile([128, NKB, H, DK + 1], BF, tag="vaug")
            qt = pp.tile([128, NI, QB], BF, tag="qt")
            msk = pp.tile([128, NKB, QB], BF, tag="msk")
            at = pp.tile([128, NI, QB], BF, tag="at")
            wo_sb = pp.tile([128, NI, D], BF, tag="wo_sb")
            xk_sb = pp.tile([128, NI, S], BF, tag="xk_sb")
            densP = [pp.tile([8, QB], FP, tag=f"densP{i}", name=f"densP{i}")
                     for i in (0, 1)]
            rdens = [pp.tile([8, QB], BF, tag=f"rdens{i}", name=f"rdens{i}")
                     for i in (0, 1)]
            bq_sb = pp.tile([128, NI], FP, tag="bq")
            bk_sb = pp.tile([128, NI], FP, tag="bk")
            bv_bc = pp.tile([128, D], BF, tag="bv")
            bo_bc = pp.tile([128, D], BF, tag="bo")
            # selector for denom broadcast: sel[k, ci, p] = 1 iff head
            # k == 2*ci + p//64; sel_ci^T @ rdens -> [128, 512] per-chunk
            # reciprocal denominators for both heads of chunk ci.
            sel = pp.tile([8, NI, 128], BF, tag="sel")
            # DMA queue budget: sync carries the V/Q/K weight + x staging
            # (needed first), scalar carries xk + mask (needed mid-kernel),
            # gpsimd SWDGE carries biases/sel/wo (small or needed late).
            nc.gpsimd.dma_start(sel[:], sel_in[:])
            nc.gpsimd.dma_start(bv_bc[:], bv_b[:])
            nc.gpsimd.dma_start(bq_sb[:], bq2[:])
            nc.gpsimd.dma_start(bk_sb[:], bk2[:])
            nc.gpsimd.dma_start(bo_bc[:], bo_b[:])
            nc.gpsimd.memset(vaug[:], 1.0)  # ones column; V cols overwritten

            for ci in range(NI):
                nc.scalar.dma_start(xk_sb[:, ci, :],
                                    xk_t[ci * 128:(ci + 1) * 128, :])
            for kb in range(NKB):
                nc.scalar.dma_start(msk[:, kb, :],
                                    mask_t[kb * 128:(kb + 1) * 128, :])
            for ci in range(NI):
                nc.gpsimd.dma_start(wo_sb[:, ci, :],
                                    wo_t[ci * 128:(ci + 1) * 128, :])

            with tc.tile_pool(name="psA", bufs=2, space="PSUM") as psA:
                # ------------- V projection (full batch) -------------
                # out[tok, vf] = xv_chunk^T @ wv ; vaug[tok, h, 0:64] strided.
                with tc.tile_pool(name="vw", bufs=1) as vwp, \
                     tc.tile_pool(name="vx", bufs=2) as vxp:
                    wv_sb = vwp.tile([128, NI, D], BF, name="wv_sb")
                    for ci in range(NI):
                        nc.sync.dma_start(wv_sb[:, ci, :],
                                          wv_t[ci * 128:(ci + 1) * 128, :])
                    for tg in range(4):  # 4 groups of 512 tokens
                        xv_sb = vxp.tile([128, NI, 512], BF, name="xv_sb")
                        for ci in range(NI):
                            nc.sync.dma_start(
                                xv_sb[:, ci, :],
                                xv_t[ci * 128:(ci + 1) * 128,
                                     tg * 512:(tg + 1) * 512])
                        for tbl in range(4):
                            tb = tg * 4 + tbl
                            for oh in range(2):
                                ps = psA.tile([128, 512], FP, name="ps")
                                for ci in range(NI):
                                    nc.tensor.matmul(
                                        ps[:],
                                        lhsT=xv_sb[:, ci,
                                                   tbl * 128:(tbl + 1) * 128],
                                        rhs=wv_sb[:, ci,
                                                  oh * 512:(oh + 1) * 512],
                                        start=(ci == 0), stop=(ci == NI - 1))
                                nc.vector.tensor_tensor(
                                    vaug[:, tb, oh * 8:(oh + 1) * 8, 0:DK],
                                    ps[:].rearrange("p (h d) -> p h d", h=8),
                                    bv_bc[:, oh * 512:(oh + 1) * 512]
                                        .rearrange("p (h d) -> p h d", h=8),
                                    OP.add)

                # ---------------- Q projection ----------------
                with tc.tile_pool(name="qstage", bufs=1) as qsp:
                    wq_sb = qsp.tile([128, NI, D], BF, name="wq_sb")
                    xq_sb = qsp.tile([128, NI, QB], BF, name="xq_sb")
                    for ci in range(NI):
                        nc.sync.dma_start(wq_sb[:, ci, :],
                                          wq_t[ci * 128:(ci + 1) * 128, :])
                        nc.sync.dma_start(xq_sb[:, ci, :],
                                          xq_t[ci * 128:(ci + 1) * 128, :])
                    for ob in range(NI):
                        ps = psA.tile([128, QB], FP, name="ps")
                        for ci in range(NI):
                            nc.tensor.matmul(
                                ps[:],
                                lhsT=wq_sb[:, ci, ob * 128:(ob + 1) * 128],
                                rhs=xq_sb[:, ci, :],
                                start=(ci == 0), stop=(ci == NI - 1))
                        nc.scalar.activation(qt[:, ob, :], ps[:], AF.Identity,
                                             bias=bq_sb[:, ob:ob + 1])

                # -------- K projection + attention, interleaved --------
                wk_sb = pp.tile([128, NI, D], BF, tag="wk_sb")
                for ci in range(NI):
                    nc.sync.dma_start(wk_sb[:, ci, :],
                                      wk_t[ci * 128:(ci + 1) * 128, :])

                k_groups = [(fb, tc4) for fb in range(NI) for tc4 in range(4)]

                def emit_k_group():
                    # one [128, fb-block] x [512 keys] accumulation group of
                    # the K projection; interleaved into the attention loop
                    # as tensor-engine filler.
                    if not k_groups:
                        return
                    fb, tc4 = k_groups.pop(0)
                    ps = psA.tile([128, 512], FP, name="ps")
                    for ci in range(NI):
                        nc.tensor.matmul(
                            ps[:],
                            lhsT=wk_sb[:, ci, fb * 128:(fb + 1) * 128],
                            rhs=xk_sb[:, ci, tc4 * 512:(tc4 + 1) * 512],
                            start=(ci == 0), stop=(ci == NI - 1))
                    ps_bc, bk_bc = bass.broadcast_tensor_aps(
                        ps[:], bk_sb[:, fb:fb + 1])
                    nc.vector.tensor_tensor(
                        kt[:, fb, tc4 * 512:(tc4 + 1) * 512],
                        ps_bc, bk_bc, OP.add)

                with tc.tile_pool(name="probs", bufs=6) as pbp, \
                     tc.tile_pool(name="dscr", bufs=3) as dsp, \
                     tc.tile_pool(name="scps", bufs=2, space="PSUM") as scp, \
                     tc.tile_pool(name="pvps", bufs=2, space="PSUM") as pvp:

                    def attn_head(h):
                        # software-pipelined emission: scores for chunk k+2
                        # are emitted before PV of chunk k so the in-order
                        # tensor engine never waits on the exp+mask latency.
                        ci_h = h // 2
                        p0 = 64 * (h % 2)
                        q_ap = qt[p0:p0 + 64, ci_h, :]
                        pv = pvp.tile([DK + 1, QB], FP, name="pv")
                        pbs = {}

                        def emit_pv(kb2):
                            pb = pbs.pop(kb2)
                            for j in range(2):
                                kb = kb2 * 2 + j
                                nc.tensor.matmul(
                                    pv[:],
                                    lhsT=vaug[:, kb, h, :],
                                    rhs=pb[:, j, :],
                                    start=(kb == 0), stop=(kb == NKB - 1))

                        for kb2 in range(NKB // 2):
                            sc = scp.tile([128, 2, 512], FP, name="sc")
                            for j in range(2):
                                kb = kb2 * 2 + j
                                nc.tensor.matmul(
                                    sc[:, j, :],
                                    lhsT=kt[p0:p0 + 64, ci_h,
                                            kb * 128:(kb + 1) * 128],
                                    rhs=q_ap,
                                    start=True, stop=True)
                            pb = pbp.tile([128, 2, 512], BF, name="pb")
                            nc.scalar.activation(pb[:], sc[:], AF.Exp,
                                                 scale=0.125)
                            nc.vector.tensor_tensor(
                                pb[:], pb[:],
                                msk[:, 2 * kb2:2 * kb2 + 2, :], OP.mult)
                            pbs[kb2] = pb
                            if kb2 in (1, 5):
                                emit_k_group()  # K-proj filler, 2 per head
                            if kb2 >= 2:
                                emit_pv(kb2 - 2)
                        emit_pv(NKB // 2 - 2)
                        emit_pv(NKB // 2 - 1)
                        nc.vector.tensor_copy(at[p0:p0 + 64, ci_h, :],
                                              pv[0:DK, :])
                        dscr = dsp.tile([1, QB], FP, name="dscr")
                        nc.vector.tensor_copy(dscr[:], pv[DK:DK + 1, :])
                        nc.sync.dma_start(dens_d[h:h + 1, :], dscr[:])

                    def normalize(half):
                        # at[:, ci, :] *= 1/denom for ci half, broadcast
                        # across partitions via a selector matmul; per-half
                        # [8, QB] tiles keep partition bases at 0.
                        h0 = half * (H // 2)
                        nc.sync.dma_start(densP[half][:],
                                          dens_d[h0:h0 + 8, :])
                        with nc.allow_low_precision(
                                reason="1/denom in bf16; denom O(1e3), "
                                       "rel err ~0.4% within tolerance"):
                            nc.vector.reciprocal(rdens[half][:],
                                                 densP[half][:])
                        for ci in range(half * 4, half * 4 + 4):
                            rbq = pvp.tile([128, QB], FP, name="pv")
                            nc.tensor.matmul(rbq[:], lhsT=sel[0:8, ci, :],
                                             rhs=rdens[half][0:8, :],
                                             start=True, stop=True)
                            nc.vector.tensor_tensor(at[:, ci, :],
                                                    at[:, ci, :],
                                                    rbq[:], OP.mult)

                    for _ in range(4):
                        emit_k_group()  # fb0 upfront for head 0
                    for h in range(H):
                        attn_head(h)
                        if h == 8:
                            normalize(0)
                    normalize(1)

                # ---------------- output projection ----------------
                with tc.tile_pool(name="outsb", bufs=2) as osp:
                    for qb in range(4):
                        osb = osp.tile([128, D], FP, name="osb")
                        for oh in range(2):
                            po = psA.tile([128, 512], FP, name="ps")
                            for ci in range(NI):
                                nc.tensor.matmul(
                                    po[:],
                                    lhsT=at[:, ci, qb * 128:(qb + 1) * 128],
                                    rhs=wo_sb[:, ci, oh * 512:(oh + 1) * 512],
                                    start=(ci == 0), stop=(ci == NI - 1))
                            nc.vector.tensor_tensor(
                                osb[:, oh * 512:(oh + 1) * 512], po[:],
                                bo_bc[:, oh * 512:(oh + 1) * 512], OP.add)
                        nc.sync.dma_start(out[qb * 128:(qb + 1) * 128, :],
                                          osb[:])
    return nc


_PATCHED = False


def _split_multi_waits(bir_bytes):
    # This walrus build allows only one sync-wait command per instruction.
    # Hoist extra waits onto EventSemaphore carriers just before each
    # instruction in the same engine stream (engines execute in order).
    import json
    j = json.loads(bir_bytes)
    for fn in j["functions"]:
        for blk in fn["blocks"]:
            out = []
            for inst in blk["instructions"]:
                si = inst.get("sync_info") or {}
                waits = si.get("on_wait") or []
                if len(waits) > 1:
                    for k, w in enumerate(waits[:-1]):
                        out.append({
                            "debug": inst.get("debug", 0),
                            "engine": inst["engine"],
                            "ins": [],
                            "name": f"{inst['name']}_w{k}",
                            "opcode": "EventSemaphore",
                            "outs": [],
                            "sync_info": {"on_update": [], "on_wait": [w]},
                        })
                    si["on_wait"] = [waits[-1]]
                out.append(inst)
            blk["instructions"] = out
    return json.dumps(j).encode()


def _patch_compiler():
    global _PATCHED
    if _PATCHED:
        return
    from concourse import bass_utils, bass2jax
    orig = bass_utils.compile_bir_kernel

    def wrapped(bir_json, tmpdir, neff_name="file.neff"):
        return orig(_split_multi_waits(bir_json), tmpdir, neff_name)

    bass_utils.compile_bir_kernel = wrapped
    bass2jax.compile_bir_kernel = wrapped
    _PATCHED = True


def kernel(query, key, value, mask, Wq, bq, Wk, bk, Wv, bv, Wo, bo):
    global LAST_EXEC_NS, LAST_RESULTS, _PROG
    _patch_compiler()
    import ml_dtypes
    from concourse.bass_utils import run_bass_kernel_spmd

    f32 = np.float32
    bf16 = ml_dtypes.bfloat16

    def tb(a):  # to bf16, C-contiguous
        return np.ascontiguousarray(np.asarray(a, dtype=f32).astype(bf16))

    wq_t = tb(np.asarray(Wq, dtype=f32).T)
    wk_t = tb(np.asarray(Wk, dtype=f32).T)
    wv_t = tb(np.asarray(Wv, dtype=f32).T)
    wo_t = tb(np.asarray(Wo, dtype=f32).T)
    bq2 = np.ascontiguousarray(np.asarray(bq, dtype=f32).reshape(NI, 128).T)
    bk2 = np.ascontiguousarray(np.asarray(bk, dtype=f32).reshape(NI, 128).T)
    bv_b = tb(np.broadcast_to(np.asarray(bv, dtype=f32).reshape(1, D),
                              (128, D)))
    bo_b = tb(np.broadcast_to(np.asarray(bo, dtype=f32).reshape(1, D),
                              (128, D)))

    sel_np = np.zeros((8, NI, 128), dtype=f32)
    for ci in range(NI):
        for hh in range(2):
            k = 2 * ci + hh - 8 * (ci >= 4)
            sel_np[k, ci, hh * 64:(hh + 1) * 64] = 1.0
    sel_np = np.ascontiguousarray(sel_np.reshape(8, NI * 128).astype(bf16))

    in_maps = []
    for r in range(NCORES):
        b, c = divmod(r, GPB)
        q0 = QB * c
        in_maps.append({
            "xq_t": tb(np.asarray(query[b, q0:q0 + QB, :], dtype=f32).T),
            "xk_t": tb(np.asarray(key[b], dtype=f32).T),
            "xv_t": tb(np.asarray(value[b], dtype=f32).T),
            "mask_t": tb(np.asarray(mask[b, q0:q0 + QB, :], dtype=f32).T),
            "wq_t": wq_t, "wk_t": wk_t, "wv_t": wv_t, "wo_t": wo_t,
            "bq2": bq2, "bk2": bk2, "bv_b": bv_b, "bo_b": bo_b,
            "sel_in": sel_np,
        })

    if _PROG is None:
        _PROG = _build()

    res = run_bass_kernel_spmd(_PROG, in_maps, core_ids=list(range(NCORES)),
                               trace=TRACE)
    LAST_EXEC_NS = res.exec_time_ns
    LAST_RESULTS = res

    out_full = np.empty((B, S, D), dtype=f32)
    for r in range(NCORES):
        b, c = divmod(r, GPB)
        q0 = QB * c
        out_full[b, q0:q0 + QB, :] = res.results[r]["out"]
    return out_full


# revision 24
# speedup vs baseline: 1.9779x; 1.0232x over previous
"""MultiHeadedAttention on 8 Trainium2 NeuronCores.

Sharding: sequence-sharded queries, replicated keys. Cores 0-3 handle
batch 0, cores 4-7 batch 1. Within a batch group, core c owns query
tokens [512c, 512c+512) but receives the FULL key/value inputs and
computes the complete K and V projections locally — no collectives,
no inter-core barrier, no AllGather on the critical path.

Everything is bf16 except PSUM accumulation, softmax denominators and
the final output (fp32). bf16 matmuls run at 1 cycle/row (vs ~1.5 for
fp32r) and enable the DVE 2x packed mode for the mask multiply.

Layouts (host-side transposes):
  kt:   [feature, key]    SBUF-resident, produced by local K projection
  qt:   [feature, query]  scores contract on the dk partitions
  vaug: [key, h, dk+1]    ones column -> PV row 64 = softmax denominator
  scores^T: [key, query]; mask applied multiplicatively after exp.

Tensor-engine emission order keeps the engine busy in-order: V proj
(full batch), Q proj, K proj feature-block 0, then attention heads with
remaining K-proj feature blocks interleaved as filler while the scalar
engine's exp stream catches up.
"""

import numpy as np

B, S, D, H, DK = 2, 2048, 1024, 16, 64
NCORES = 8
GPB = 4             # cores per batch group
QB = S // GPB       # 512 query tokens per core
NI = D // 128       # 8 feature chunks
NKB = S // 128      # 16 key blocks

TRACE = False
LAST_EXEC_NS = None
LAST_RESULTS = None
_PROG = None


def _build():
    from concourse import bass, mybir, tile

    FP = mybir.dt.float32
    BF = mybir.dt.bfloat16
    AF = mybir.ActivationFunctionType
    OP = mybir.AluOpType

    nc = bass.Bass(num_devices=NCORES)

    xq_t = nc.dram_tensor("xq_t", [D, QB], BF, kind="ExternalInput")
    xk_t = nc.dram_tensor("xk_t", [D, S], BF, kind="ExternalInput")
    xv_t = nc.dram_tensor("xv_t", [D, S], BF, kind="ExternalInput")
    mask_t = nc.dram_tensor("mask_t", [S, QB], BF, kind="ExternalInput")
    wq_t = nc.dram_tensor("wq_t", [D, D], BF, kind="ExternalInput")
    wk_t = nc.dram_tensor("wk_t", [D, D], BF, kind="ExternalInput")
    wv_t = nc.dram_tensor("wv_t", [D, D], BF, kind="ExternalInput")
    wo_t = nc.dram_tensor("wo_t", [D, D], BF, kind="ExternalInput")
    bq2 = nc.dram_tensor("bq2", [128, NI], FP, kind="ExternalInput")
    bk2 = nc.dram_tensor("bk2", [128, NI], FP, kind="ExternalInput")
    bv_b = nc.dram_tensor("bv_b", [128, D], BF, kind="ExternalInput")
    bo_b = nc.dram_tensor("bo_b", [128, D], BF, kind="ExternalInput")
    sel_in = nc.dram_tensor("sel_in", [8, NI * 128], BF, kind="ExternalInput")
    out = nc.dram_tensor("out", [QB, D], FP, kind="ExternalOutput")

    with tile.TileContext(nc) as tc:
        with tc.tile_pool(name="dram", bufs=1, space="DRAM") as dpool, \
             tc.tile_pool(name="persist", bufs=1) as pp:
            dens_d = dpool.tile([H, QB], FP, tag="dens_d")
            kt = pp.tile([128, NI, S], BF, tag="kt")
            vaug = pp.tile([128, NKB, H, DK + 1], BF, tag="vaug")
            qt = pp.tile([128, NI, QB], BF, tag="qt")
            msk = pp.tile([128, NKB, QB], BF, tag="msk")
            at = pp.tile([128, NI, QB], BF, tag="at")
            wo_sb = pp.tile([128, NI, D], BF, tag="wo_sb")
            xk_sb = pp.tile([128, NI, S], BF, tag="xk_sb")
            densP = [pp.tile([8, QB], FP, tag=f"densP{i}", name=f"densP{i}")
                     for i in (0, 1)]
            rdens = [pp.tile([8, QB], BF, tag=f"rdens{i}", name=f"rdens{i}")
                     for i in (0, 1)]
            bq_sb = pp.tile([128, NI], FP, tag="bq")
            bk_sb = pp.tile([128, NI], FP, tag="bk")
            bv_bc = pp.tile([128, D], BF, tag="bv")
            bo_bc = pp.tile([128, D], BF, tag="bo")
            # selector for denom broadcast: sel[k, ci, p] = 1 iff head
            # k == 2*ci + p//64; sel_ci^T @ rdens -> [128, 512] per-chunk
            # reciprocal denominators for both heads of chunk ci.
            sel = pp.tile([8, NI, 128], BF, tag="sel")
            # DMA queue budget: sync carries the V/Q/K weight + x staging
            # (needed first), scalar carries xk + mask (needed mid-kernel),
            # gpsimd SWDGE carries biases/sel/wo (small or needed late).
            nc.gpsimd.dma_start(sel[:], sel_in[:])
            nc.gpsimd.dma_start(bv_bc[:], bv_b[:])
            nc.gpsimd.dma_start(bq_sb[:], bq2[:])
            nc.gpsimd.dma_start(bk_sb[:], bk2[:])
            nc.gpsimd.dma_start(bo_bc[:], bo_b[:])
            nc.gpsimd.memset(vaug[:], 1.0)  # ones column; V cols overwritten

            for ci in range(NI):
                nc.gpsimd.dma_start(wo_sb[:, ci, :],
                                    wo_t[ci * 128:(ci + 1) * 128, :])

            with tc.tile_pool(name="psA", bufs=2, space="PSUM") as psA:
                # ------------- V projection (full batch) -------------
                # out[tok, vf] = xv_chunk^T @ wv ; vaug[tok, h, 0:64] strided.
                with tc.tile_pool(name="vw", bufs=1) as vwp, \
                     tc.tile_pool(name="vx", bufs=2) as vxp:
                    wv_sb = vwp.tile([128, NI, D], BF, name="wv_sb")
                    for ci in range(NI):
                        nc.scalar.dma_start(wv_sb[:, ci, :],
                                            wv_t[ci * 128:(ci + 1) * 128, :])
                    # xk + mask ride the scalar queue behind wv; they are
                    # needed only once attention starts (~90us in).
                    for ci in range(NI):
                        nc.scalar.dma_start(xk_sb[:, ci, :],
                                            xk_t[ci * 128:(ci + 1) * 128, :])
                    for kb in range(NKB):
                        nc.scalar.dma_start(msk[:, kb, :],
                                            mask_t[kb * 128:(kb + 1) * 128, :])
                    for tg in range(4):  # 4 groups of 512 tokens
                        xv_sb = vxp.tile([128, NI, 512], BF, name="xv_sb")
                        for ci in range(NI):
                            nc.sync.dma_start(
                                xv_sb[:, ci, :],
                                xv_t[ci * 128:(ci + 1) * 128,
                                     tg * 512:(tg + 1) * 512])
                        for tbl in range(4):
                            tb = tg * 4 + tbl
                            for oh in range(2):
                                ps = psA.tile([128, 512], FP, name="ps")
                                for ci in range(NI):
                                    nc.tensor.matmul(
                                        ps[:],
                                        lhsT=xv_sb[:, ci,
                                                   tbl * 128:(tbl + 1) * 128],
                                        rhs=wv_sb[:, ci,
                                                  oh * 512:(oh + 1) * 512],
                                        start=(ci == 0), stop=(ci == NI - 1))
                                nc.vector.tensor_tensor(
                                    vaug[:, tb, oh * 8:(oh + 1) * 8, 0:DK],
                                    ps[:].rearrange("p (h d) -> p h d", h=8),
                                    bv_bc[:, oh * 512:(oh + 1) * 512]
                                        .rearrange("p (h d) -> p h d", h=8),
                                    OP.add)

                # ---------------- Q projection ----------------
                with tc.tile_pool(name="qstage", bufs=1) as qsp:
                    wq_sb = qsp.tile([128, NI, D], BF, name="wq_sb")
                    xq_sb = qsp.tile([128, NI, QB], BF, name="xq_sb")
                    for ci in range(NI):
                        nc.sync.dma_start(wq_sb[:, ci, :],
                                          wq_t[ci * 128:(ci + 1) * 128, :])
                        nc.sync.dma_start(xq_sb[:, ci, :],
                                          xq_t[ci * 128:(ci + 1) * 128, :])
                    for ob in range(NI):
                        ps = psA.tile([128, QB], FP, name="ps")
                        for ci in range(NI):
                            nc.tensor.matmul(
                                ps[:],
                                lhsT=wq_sb[:, ci, ob * 128:(ob + 1) * 128],
                                rhs=xq_sb[:, ci, :],
                                start=(ci == 0), stop=(ci == NI - 1))
                        nc.scalar.activation(qt[:, ob, :], ps[:], AF.Identity,
                                             bias=bq_sb[:, ob:ob + 1])

                # -------- K projection + attention, interleaved --------
                wk_sb = pp.tile([128, NI, D], BF, tag="wk_sb")
                for ci in range(NI):
                    nc.sync.dma_start(wk_sb[:, ci, :],
                                      wk_t[ci * 128:(ci + 1) * 128, :])

                k_groups = [(fb, tc4) for fb in range(NI) for tc4 in range(4)]

                def emit_k_group():
                    # one [128, fb-block] x [512 keys] accumulation group of
                    # the K projection; interleaved into the attention loop
                    # as tensor-engine filler.
                    if not k_groups:
                        return
                    fb, tc4 = k_groups.pop(0)
                    ps = psA.tile([128, 512], FP, name="ps")
                    for ci in range(NI):
                        nc.tensor.matmul(
                            ps[:],
                            lhsT=wk_sb[:, ci, fb * 128:(fb + 1) * 128],
                            rhs=xk_sb[:, ci, tc4 * 512:(tc4 + 1) * 512],
                            start=(ci == 0), stop=(ci == NI - 1))
                    ps_bc, bk_bc = bass.broadcast_tensor_aps(
                        ps[:], bk_sb[:, fb:fb + 1])
                    nc.vector.tensor_tensor(
                        kt[:, fb, tc4 * 512:(tc4 + 1) * 512],
                        ps_bc, bk_bc, OP.add)

                with tc.tile_pool(name="probs", bufs=6) as pbp, \
                     tc.tile_pool(name="dscr", bufs=3) as dsp, \
                     tc.tile_pool(name="scps", bufs=2, space="PSUM") as scp, \
                     tc.tile_pool(name="pvps", bufs=2, space="PSUM") as pvp:

                    def attn_pair(fb):
                        # two heads (2fb, 2fb+1) interleaved per key chunk:
                        # the scores->exp->mask->PV chain of each head hides
                        # behind the other head's tensor work, so the
                        # in-order tensor engine rarely waits. PV lags the
                        # scores by one chunk step.
                        heads = (2 * fb, 2 * fb + 1)
                        pvs = {h: pvp.tile([DK + 1, QB], FP, name="pv")
                               for h in heads}
                        pbs = {}

                        def emit_pv(h, kb2):
                            pb = pbs.pop((h, kb2))
                            for j in range(2):
                                kb = kb2 * 2 + j
                                nc.tensor.matmul(
                                    pvs[h][:],
                                    lhsT=vaug[:, kb, h, :],
                                    rhs=pb[:, j, :],
                                    start=(kb == 0), stop=(kb == NKB - 1))

                        for kb2 in range(NKB // 2):
                            for h in heads:
                                p0 = 64 * (h % 2)
                                sc = scp.tile([128, 2, 512], FP, name="sc")
                                for j in range(2):
                                    kb = kb2 * 2 + j
                                    nc.tensor.matmul(
                                        sc[:, j, :],
                                        lhsT=kt[p0:p0 + 64, fb,
                                                kb * 128:(kb + 1) * 128],
                                        rhs=qt[p0:p0 + 64, fb, :],
                                        start=True, stop=True)
                                pb = pbp.tile([128, 2, 512], BF, name="pb")
                                nc.scalar.activation(pb[:], sc[:], AF.Exp,
                                                     scale=0.125)
                                nc.vector.tensor_tensor(
                                    pb[:], pb[:],
                                    msk[:, 2 * kb2:2 * kb2 + 2, :], OP.mult)
                                pbs[(h, kb2)] = pb
                            if kb2 % 2 == 1:
                                emit_k_group()  # K-proj filler, 4 per pair
                            if kb2 >= 1:
                                for h in heads:
                                    emit_pv(h, kb2 - 1)
                        for h in heads:
                            emit_pv(h, NKB // 2 - 1)
                        for h in heads:
                            p0 = 64 * (h % 2)
                            nc.vector.tensor_copy(at[p0:p0 + 64, fb, :],
                                                  pvs[h][0:DK, :])
                            dscr = dsp.tile([1, QB], FP, name="dscr")
                            nc.vector.tensor_copy(dscr[:], pvs[h][DK:DK + 1, :])
                            nc.scalar.dma_start(dens_d[h:h + 1, :], dscr[:])

                    def normalize(half):
                        # at[:, ci, :] *= 1/denom for ci half, broadcast
                        # across partitions via a selector matmul; per-half
                        # [8, QB] tiles keep partition bases at 0.
                        h0 = half * (H // 2)
                        nc.scalar.dma_start(densP[half][:],
                                            dens_d[h0:h0 + 8, :])
                        with nc.allow_low_precision(
                                reason="1/denom in bf16; denom O(1e3), "
                                       "rel err ~0.4% within tolerance"):
                            nc.vector.reciprocal(rdens[half][:],
                                                 densP[half][:])
                        for ci in range(half * 4, half * 4 + 4):
                            rbq = pvp.tile([128, QB], FP, name="pv")
                            nc.tensor.matmul(rbq[:], lhsT=sel[0:8, ci, :],
                                             rhs=rdens[half][0:8, :],
                                             start=True, stop=True)
                            nc.vector.tensor_tensor(at[:, ci, :],
                                                    at[:, ci, :],
                                                    rbq[:], OP.mult)

                    for _ in range(4):
                        emit_k_group()  # fb0 upfront for pair 0
                    for fb in range(NI):
                        attn_pair(fb)
                        if fb == 4:
                            normalize(0)
                    normalize(1)

                # ---------------- output projection ----------------
                with tc.tile_pool(name="outsb", bufs=2) as osp:
                    for qb in range(4):
                        osb = osp.tile([128, D], FP, name="osb")
                        for oh in range(2):
                            po = psA.tile([128, 512], FP, name="ps")
                            for ci in range(NI):
                                nc.tensor.matmul(
                                    po[:],
                                    lhsT=at[:, ci, qb * 128:(qb + 1) * 128],
                                    rhs=wo_sb[:, ci, oh * 512:(oh + 1) * 512],
                                    start=(ci == 0), stop=(ci == NI - 1))
                            nc.vector.tensor_tensor(
                                osb[:, oh * 512:(oh + 1) * 512], po[:],
                                bo_bc[:, oh * 512:(oh + 1) * 512], OP.add)
                        nc.sync.dma_start(out[qb * 128:(qb + 1) * 128, :],
                                          osb[:])
    return nc


_PATCHED = False


def _split_multi_waits(bir_bytes):
    # This walrus build allows only one sync-wait command per instruction.
    # Hoist extra waits onto EventSemaphore carriers just before each
    # instruction in the same engine stream (engines execute in order).
    import json
    j = json.loads(bir_bytes)
    for fn in j["functions"]:
        for blk in fn["blocks"]:
            out = []
            for inst in blk["instructions"]:
                si = inst.get("sync_info") or {}
                waits = si.get("on_wait") or []
                if len(waits) > 1:
                    for k, w in enumerate(waits[:-1]):
                        out.append({
                            "debug": inst.get("debug", 0),
                            "engine": inst["engine"],
                            "ins": [],
                            "name": f"{inst['name']}_w{k}",
                            "opcode": "EventSemaphore",
                            "outs": [],
                            "sync_info": {"on_update": [], "on_wait": [w]},
                        })
                    si["on_wait"] = [waits[-1]]
                out.append(inst)
            blk["instructions"] = out
    return json.dumps(j).encode()


def _patch_compiler():
    global _PATCHED
    if _PATCHED:
        return
    from concourse import bass_utils, bass2jax
    orig = bass_utils.compile_bir_kernel

    def wrapped(bir_json, tmpdir, neff_name="file.neff"):
        return orig(_split_multi_waits(bir_json), tmpdir, neff_name)

    bass_utils.compile_bir_kernel = wrapped
    bass2jax.compile_bir_kernel = wrapped
    _PATCHED = True


def kernel(query, key, value, mask, Wq, bq, Wk, bk, Wv, bv, Wo, bo):
    global LAST_EXEC_NS, LAST_RESULTS, _PROG
    _patch_compiler()
    import ml_dtypes
    from concourse.bass_utils import run_bass_kernel_spmd

    f32 = np.float32
    bf16 = ml_dtypes.bfloat16

    def tb(a):  # to bf16, C-contiguous
        return np.ascontiguousarray(np.asarray(a, dtype=f32).astype(bf16))

    wq_t = tb(np.asarray(Wq, dtype=f32).T)
    wk_t = tb(np.asarray(Wk, dtype=f32).T)
    wv_t = tb(np.asarray(Wv, dtype=f32).T)
    wo_t = tb(np.asarray(Wo, dtype=f32).T)
    bq2 = np.ascontiguousarray(np.asarray(bq, dtype=f32).reshape(NI, 128).T)
    bk2 = np.ascontiguousarray(np.asarray(bk, dtype=f32).reshape(NI, 128).T)
    bv_b = tb(np.broadcast_to(np.asarray(bv, dtype=f32).reshape(1, D),
                              (128, D)))
    bo_b = tb(np.broadcast_to(np.asarray(bo, dtype=f32).reshape(1, D),
                              (128, D)))

    sel_np = np.zeros((8, NI, 128), dtype=f32)
    for ci in range(NI):
        for hh in range(2):
            k = 2 * ci + hh - 8 * (ci >= 4)
            sel_np[k, ci, hh * 64:(hh + 1) * 64] = 1.0
    sel_np = np.ascontiguousarray(sel_np.reshape(8, NI * 128).astype(bf16))

    in_maps = []
    for r in range(NCORES):
        b, c = divmod(r, GPB)
        q0 = QB * c
        in_maps.append({
            "xq_t": tb(np.asarray(query[b, q0:q0 + QB, :], dtype=f32).T),
            "xk_t": tb(np.asarray(key[b], dtype=f32).T),
            "xv_t": tb(np.asarray(value[b], dtype=f32).T),
            "mask_t": tb(np.asarray(mask[b, q0:q0 + QB, :], dtype=f32).T),
            "wq_t": wq_t, "wk_t": wk_t, "wv_t": wv_t, "wo_t": wo_t,
            "bq2": bq2, "bk2": bk2, "bv_b": bv_b, "bo_b": bo_b,
            "sel_in": sel_np,
        })

    if _PROG is None:
        _PROG = _build()

    res = run_bass_kernel_spmd(_PROG, in_maps, core_ids=list(range(NCORES)),
                               trace=TRACE)
    LAST_EXEC_NS = res.exec_time_ns
    LAST_RESULTS = res

    out_full = np.empty((B, S, D), dtype=f32)
    for r in range(NCORES):
        b, c = divmod(r, GPB)
        q0 = QB * c
        out_full[b, q0:q0 + QB, :] = res.results[r]["out"]
    return out_full


# revision 25
# speedup vs baseline: 1.9903x; 1.0063x over previous
"""MultiHeadedAttention on 8 Trainium2 NeuronCores.

Sharding: sequence-sharded queries, replicated keys. Cores 0-3 handle
batch 0, cores 4-7 batch 1. Within a batch group, core c owns query
tokens [512c, 512c+512) but receives the FULL key/value inputs and
computes the complete K and V projections locally — no collectives,
no inter-core barrier, no AllGather on the critical path.

Everything is bf16 except PSUM accumulation, softmax denominators and
the final output (fp32). bf16 matmuls run at 1 cycle/row (vs ~1.5 for
fp32r) and enable the DVE 2x packed mode for the mask multiply.

Layouts (host-side transposes):
  kt:   [feature, key]    SBUF-resident, produced by local K projection
  qt:   [feature, query]  scores contract on the dk partitions
  vaug: [key, h, dk+1]    ones column -> PV row 64 = softmax denominator
  scores^T: [key, query]; mask applied multiplicatively after exp.

Tensor-engine emission order keeps the engine busy in-order: V proj
(full batch), Q proj, K proj feature-block 0, then attention heads with
remaining K-proj feature blocks interleaved as filler while the scalar
engine's exp stream catches up.
"""

import numpy as np

B, S, D, H, DK = 2, 2048, 1024, 16, 64
NCORES = 8
GPB = 4             # cores per batch group
QB = S // GPB       # 512 query tokens per core
NI = D // 128       # 8 feature chunks
NKB = S // 128      # 16 key blocks

TRACE = False
LAST_EXEC_NS = None
LAST_RESULTS = None
_PROG = None


def _build():
    from concourse import bass, mybir, tile

    FP = mybir.dt.float32
    BF = mybir.dt.bfloat16
    AF = mybir.ActivationFunctionType
    OP = mybir.AluOpType

    nc = bass.Bass(num_devices=NCORES)

    xq_t = nc.dram_tensor("xq_t", [D, QB], BF, kind="ExternalInput")
    xk_t = nc.dram_tensor("xk_t", [D, S], BF, kind="ExternalInput")
    xv_t = nc.dram_tensor("xv_t", [D, S], BF, kind="ExternalInput")
    mask_t = nc.dram_tensor("mask_t", [S, QB], BF, kind="ExternalInput")
    wq_t = nc.dram_tensor("wq_t", [D, D], BF, kind="ExternalInput")
    wk_t = nc.dram_tensor("wk_t", [D, D], BF, kind="ExternalInput")
    wv_t = nc.dram_tensor("wv_t", [D, D], BF, kind="ExternalInput")
    wo_t = nc.dram_tensor("wo_t", [D, D], BF, kind="ExternalInput")
    bq2 = nc.dram_tensor("bq2", [128, NI], FP, kind="ExternalInput")
    bk2 = nc.dram_tensor("bk2", [128, NI], FP, kind="ExternalInput")
    bv_b = nc.dram_tensor("bv_b", [128, D], BF, kind="ExternalInput")
    bo_b = nc.dram_tensor("bo_b", [128, D], BF, kind="ExternalInput")
    sel_in = nc.dram_tensor("sel_in", [8, NI * 128], BF, kind="ExternalInput")
    out = nc.dram_tensor("out", [QB, D], FP, kind="ExternalOutput")

    with tile.TileContext(nc) as tc:
        with tc.tile_pool(name="dram", bufs=1, space="DRAM") as dpool, \
             tc.tile_pool(name="persist", bufs=1) as pp:
            dens_d = dpool.tile([H, QB], FP, tag="dens_d")
            kt = pp.tile([128, NI, S], BF, tag="kt")
            vaug = pp.tile([128, NKB, H, DK + 1], BF, tag="vaug")
            qt = pp.tile([128, NI, QB], BF, tag="qt")
            msk = pp.tile([128, NKB, QB], BF, tag="msk")
            at = pp.tile([128, NI, QB], BF, tag="at")
            wo_sb = pp.tile([128, NI, D], BF, tag="wo_sb")
            xk_sb = pp.tile([128, NI, S], BF, tag="xk_sb")
            densP = [pp.tile([8, QB], FP, tag=f"densP{i}", name=f"densP{i}")
                     for i in (0, 1)]
            rdens = [pp.tile([8, QB], BF, tag=f"rdens{i}", name=f"rdens{i}")
                     for i in (0, 1)]
            bq_sb = pp.tile([128, NI], FP, tag="bq")
            bk_sb = pp.tile([128, NI], FP, tag="bk")
            bv_bc = pp.tile([128, D], BF, tag="bv")
            bo_bc = pp.tile([128, D], BF, tag="bo")
            # selector for denom broadcast: sel[k, ci, p] = 1 iff head
            # k == 2*ci + p//64; sel_ci^T @ rdens -> [128, 512] per-chunk
            # reciprocal denominators for both heads of chunk ci.
            sel = pp.tile([8, NI, 128], BF, tag="sel")
            # DMA queue budget: sync carries the V/Q/K weight + x staging
            # (needed first), scalar carries xk + mask (needed mid-kernel),
            # gpsimd SWDGE carries biases/sel/wo (small or needed late).
            nc.gpsimd.dma_start(sel[:], sel_in[:])
            nc.gpsimd.dma_start(bv_bc[:], bv_b[:])
            nc.gpsimd.dma_start(bq_sb[:], bq2[:])
            nc.gpsimd.dma_start(bk_sb[:], bk2[:])
            nc.gpsimd.dma_start(bo_bc[:], bo_b[:])
            nc.gpsimd.memset(vaug[:], 1.0)  # ones column; V cols overwritten
            # preload the exp activation table set while ScalarE is idle so
            # the first real exp doesn't pay the ~2.7us table load.
            warm = pp.tile([1, 16], FP, tag="warm")
            nc.gpsimd.memset(warm[:], 0.0)
            nc.scalar.activation(warm[:], warm[:], AF.Exp)

            for ci in range(NI):
                nc.gpsimd.dma_start(wo_sb[:, ci, :],
                                    wo_t[ci * 128:(ci + 1) * 128, :])

            with tc.tile_pool(name="psA", bufs=2, space="PSUM") as psA:
                # ------------- V projection (full batch) -------------
                # out[tok, vf] = xv_chunk^T @ wv ; vaug[tok, h, 0:64] strided.
                with tc.tile_pool(name="vw", bufs=1) as vwp, \
                     tc.tile_pool(name="vx", bufs=2) as vxp:
                    wv_sb = vwp.tile([128, NI, D], BF, name="wv_sb")
                    for ci in range(NI):
                        nc.scalar.dma_start(wv_sb[:, ci, :],
                                            wv_t[ci * 128:(ci + 1) * 128, :])
                    for tg in range(4):  # 4 groups of 512 tokens
                        xv_sb = vxp.tile([128, NI, 512], BF, name="xv_sb")
                        eng = nc.sync if tg % 2 == 0 else nc.scalar
                        for ci in range(NI):
                            eng.dma_start(
                                xv_sb[:, ci, :],
                                xv_t[ci * 128:(ci + 1) * 128,
                                     tg * 512:(tg + 1) * 512])
                        for tbl in range(4):
                            tb = tg * 4 + tbl
                            for oh in range(2):
                                ps = psA.tile([128, 512], FP, name="ps")
                                for ci in range(NI):
                                    nc.tensor.matmul(
                                        ps[:],
                                        lhsT=xv_sb[:, ci,
                                                   tbl * 128:(tbl + 1) * 128],
                                        rhs=wv_sb[:, ci,
                                                  oh * 512:(oh + 1) * 512],
                                        start=(ci == 0), stop=(ci == NI - 1))
                                nc.vector.tensor_tensor(
                                    vaug[:, tb, oh * 8:(oh + 1) * 8, 0:DK],
                                    ps[:].rearrange("p (h d) -> p h d", h=8),
                                    bv_bc[:, oh * 512:(oh + 1) * 512]
                                        .rearrange("p (h d) -> p h d", h=8),
                                    OP.add)

                # xk + mask ride the scalar queue after wv/xv; they are
                # needed only once attention starts (~90us in).
                for ci in range(NI):
                    nc.scalar.dma_start(xk_sb[:, ci, :],
                                        xk_t[ci * 128:(ci + 1) * 128, :])
                for kb in range(NKB):
                    nc.scalar.dma_start(msk[:, kb, :],
                                        mask_t[kb * 128:(kb + 1) * 128, :])

                # ---------------- Q projection ----------------
                with tc.tile_pool(name="qstage", bufs=1) as qsp:
                    wq_sb = qsp.tile([128, NI, D], BF, name="wq_sb")
                    xq_sb = qsp.tile([128, NI, QB], BF, name="xq_sb")
                    for ci in range(NI):
                        nc.sync.dma_start(wq_sb[:, ci, :],
                                          wq_t[ci * 128:(ci + 1) * 128, :])
                        nc.sync.dma_start(xq_sb[:, ci, :],
                                          xq_t[ci * 128:(ci + 1) * 128, :])
                    for ob in range(NI):
                        ps = psA.tile([128, QB], FP, name="ps")
                        for ci in range(NI):
                            nc.tensor.matmul(
                                ps[:],
                                lhsT=wq_sb[:, ci, ob * 128:(ob + 1) * 128],
                                rhs=xq_sb[:, ci, :],
                                start=(ci == 0), stop=(ci == NI - 1))
                        nc.scalar.activation(qt[:, ob, :], ps[:], AF.Identity,
                                             bias=bq_sb[:, ob:ob + 1])

                # -------- K projection + attention, interleaved --------
                wk_sb = pp.tile([128, NI, D], BF, tag="wk_sb")
                for ci in range(NI):
                    nc.sync.dma_start(wk_sb[:, ci, :],
                                      wk_t[ci * 128:(ci + 1) * 128, :])

                k_groups = [(fb, tc4) for fb in range(NI) for tc4 in range(4)]

                def emit_k_group():
                    # one [128, fb-block] x [512 keys] accumulation group of
                    # the K projection; interleaved into the attention loop
                    # as tensor-engine filler.
                    if not k_groups:
                        return
                    fb, tc4 = k_groups.pop(0)
                    ps = psA.tile([128, 512], FP, name="ps")
                    for ci in range(NI):
                        nc.tensor.matmul(
                            ps[:],
                            lhsT=wk_sb[:, ci, fb * 128:(fb + 1) * 128],
                            rhs=xk_sb[:, ci, tc4 * 512:(tc4 + 1) * 512],
                            start=(ci == 0), stop=(ci == NI - 1))
                    ps_bc, bk_bc = bass.broadcast_tensor_aps(
                        ps[:], bk_sb[:, fb:fb + 1])
                    nc.vector.tensor_tensor(
                        kt[:, fb, tc4 * 512:(tc4 + 1) * 512],
                        ps_bc, bk_bc, OP.add)

                with tc.tile_pool(name="probs", bufs=6) as pbp, \
                     tc.tile_pool(name="dscr", bufs=3) as dsp, \
                     tc.tile_pool(name="scps", bufs=2, space="PSUM") as scp, \
                     tc.tile_pool(name="pvps", bufs=2, space="PSUM") as pvp:

                    def attn_pair(fb):
                        # two heads (2fb, 2fb+1) interleaved per key chunk:
                        # the scores->exp->mask->PV chain of each head hides
                        # behind the other head's tensor work, so the
                        # in-order tensor engine rarely waits. PV lags the
                        # scores by one chunk step.
                        heads = (2 * fb, 2 * fb + 1)
                        pvs = {h: pvp.tile([DK + 1, QB], FP, name="pv")
                               for h in heads}
                        pbs = {}

                        def emit_pv(h, kb2):
                            pb = pbs.pop((h, kb2))
                            for j in range(2):
                                kb = kb2 * 2 + j
                                nc.tensor.matmul(
                                    pvs[h][:],
                                    lhsT=vaug[:, kb, h, :],
                                    rhs=pb[:, j, :],
                                    start=(kb == 0), stop=(kb == NKB - 1))

                        for kb2 in range(NKB // 2):
                            for h in heads:
                                p0 = 64 * (h % 2)
                                sc = scp.tile([128, 2, 512], FP, name="sc")
                                for j in range(2):
                                    kb = kb2 * 2 + j
                                    nc.tensor.matmul(
                                        sc[:, j, :],
                                        lhsT=kt[p0:p0 + 64, fb,
                                                kb * 128:(kb + 1) * 128],
                                        rhs=qt[p0:p0 + 64, fb, :],
                                        start=True, stop=True)
                                pb = pbp.tile([128, 2, 512], BF, name="pb")
                                nc.scalar.activation(pb[:], sc[:], AF.Exp,
                                                     scale=0.125)
                                nc.vector.tensor_tensor(
                                    pb[:], pb[:],
                                    msk[:, 2 * kb2:2 * kb2 + 2, :], OP.mult)
                                pbs[(h, kb2)] = pb
                            if kb2 % 2 == 1:
                                emit_k_group()  # K-proj filler, 4 per pair
                            if kb2 >= 1:
                                for h in heads:
                                    emit_pv(h, kb2 - 1)
                        for h in heads:
                            emit_pv(h, NKB // 2 - 1)
                        for h in heads:
                            p0 = 64 * (h % 2)
                            nc.vector.tensor_copy(at[p0:p0 + 64, fb, :],
                                                  pvs[h][0:DK, :])
                            dscr = dsp.tile([1, QB], FP, name="dscr")
                            nc.vector.tensor_copy(dscr[:], pvs[h][DK:DK + 1, :])
                            nc.scalar.dma_start(dens_d[h:h + 1, :], dscr[:])

                    def normalize(half, pregathered=0):
                        # at[:, ci, :] *= 1/denom for ci half, broadcast
                        # across partitions via a selector matmul; per-half
                        # [8, QB] tiles keep partition bases at 0.
                        h0 = half * (H // 2)
                        nc.scalar.dma_start(densP[half][pregathered:8, :],
                                            dens_d[h0 + pregathered:
                                                   h0 + 8, :])
                        with nc.allow_low_precision(
                                reason="1/denom in bf16; denom O(1e3), "
                                       "rel err ~0.4% within tolerance"):
                            nc.vector.reciprocal(rdens[half][:],
                                                 densP[half][:])
                        for ci in range(half * 4, half * 4 + 4):
                            rbq = pvp.tile([128, QB], FP, name="pv")
                            nc.tensor.matmul(rbq[:], lhsT=sel[0:8, ci, :],
                                             rhs=rdens[half][0:8, :],
                                             start=True, stop=True)
                            nc.vector.tensor_tensor(at[:, ci, :],
                                                    at[:, ci, :],
                                                    rbq[:], OP.mult)

                    for _ in range(4):
                        emit_k_group()  # fb0 upfront for pair 0
                    for fb in range(NI):
                        attn_pair(fb)
                        if fb == 4:
                            normalize(0)
                        if fb == 6:
                            # pre-gather heads 8..13 so the final normalize
                            # only waits on the last pair's two rows
                            nc.scalar.dma_start(densP[1][0:6, :],
                                                dens_d[8:14, :])
                    normalize(1, pregathered=6)

                # ---------------- output projection ----------------
                with tc.tile_pool(name="outsb", bufs=2) as osp:
                    for qb in range(4):
                        osb = osp.tile([128, D], FP, name="osb")
                        for oh in range(2):
                            po = psA.tile([128, 512], FP, name="ps")
                            for ci in range(NI):
                                nc.tensor.matmul(
                                    po[:],
                                    lhsT=at[:, ci, qb * 128:(qb + 1) * 128],
                                    rhs=wo_sb[:, ci, oh * 512:(oh + 1) * 512],
                                    start=(ci == 0), stop=(ci == NI - 1))
                            nc.vector.tensor_tensor(
                                osb[:, oh * 512:(oh + 1) * 512], po[:],
                                bo_bc[:, oh * 512:(oh + 1) * 512], OP.add)
                        nc.sync.dma_start(out[qb * 128:(qb + 1) * 128, :],
                                          osb[:])
    return nc


_PATCHED = False


def _split_multi_waits(bir_bytes):
    # This walrus build allows only one sync-wait command per instruction.
    # Hoist extra waits onto EventSemaphore carriers just before each
    # instruction in the same engine stream (engines execute in order).
    import json
    j = json.loads(bir_bytes)
    for fn in j["functions"]:
        for blk in fn["blocks"]:
            out = []
            for inst in blk["instructions"]:
                si = inst.get("sync_info") or {}
                waits = si.get("on_wait") or []
                if len(waits) > 1:
                    for k, w in enumerate(waits[:-1]):
                        out.append({
                            "debug": inst.get("debug", 0),
                            "engine": inst["engine"],
                            "ins": [],
                            "name": f"{inst['name']}_w{k}",
                            "opcode": "EventSemaphore",
                            "outs": [],
                            "sync_info": {"on_update": [], "on_wait": [w]},
                        })
                    si["on_wait"] = [waits[-1]]
                out.append(inst)
            blk["instructions"] = out
    return json.dumps(j).encode()


def _patch_compiler():
    global _PATCHED
    if _PATCHED:
        return
    from concourse import bass_utils, bass2jax
    orig = bass_utils.compile_bir_kernel

    def wrapped(bir_json, tmpdir, neff_name="file.neff"):
        return orig(_split_multi_waits(bir_json), tmpdir, neff_name)

    bass_utils.compile_bir_kernel = wrapped
    bass2jax.compile_bir_kernel = wrapped
    _PATCHED = True


def kernel(query, key, value, mask, Wq, bq, Wk, bk, Wv, bv, Wo, bo):
    global LAST_EXEC_NS, LAST_RESULTS, _PROG
    _patch_compiler()
    import ml_dtypes
    from concourse.bass_utils import run_bass_kernel_spmd

    f32 = np.float32
    bf16 = ml_dtypes.bfloat16

    def tb(a):  # to bf16, C-contiguous
        return np.ascontiguousarray(np.asarray(a, dtype=f32).astype(bf16))

    wq_t = tb(np.asarray(Wq, dtype=f32).T)
    wk_t = tb(np.asarray(Wk, dtype=f32).T)
    wv_t = tb(np.asarray(Wv, dtype=f32).T)
    wo_t = tb(np.asarray(Wo, dtype=f32).T)
    bq2 = np.ascontiguousarray(np.asarray(bq, dtype=f32).reshape(NI, 128).T)
    bk2 = np.ascontiguousarray(np.asarray(bk, dtype=f32).reshape(NI, 128).T)
    bv_b = tb(np.broadcast_to(np.asarray(bv, dtype=f32).reshape(1, D),
                              (128, D)))
    bo_b = tb(np.broadcast_to(np.asarray(bo, dtype=f32).reshape(1, D),
                              (128, D)))

    sel_np = np.zeros((8, NI, 128), dtype=f32)
    for ci in range(NI):
        for hh in range(2):
            k = 2 * ci + hh - 8 * (ci >= 4)
            sel_np[k, ci, hh * 64:(hh + 1) * 64] = 1.0
    sel_np = np.ascontiguousarray(sel_np.reshape(8, NI * 128).astype(bf16))

    in_maps = []
    for r in range(NCORES):
        b, c = divmod(r, GPB)
        q0 = QB * c
        in_maps.append({
            "xq_t": tb(np.asarray(query[b, q0:q0 + QB, :], dtype=f32).T),
            "xk_t": tb(np.asarray(key[b], dtype=f32).T),
            "xv_t": tb(np.asarray(value[b], dtype=f32).T),
            "mask_t": tb(np.asarray(mask[b, q0:q0 + QB, :], dtype=f32).T),
            "wq_t": wq_t, "wk_t": wk_t, "wv_t": wv_t, "wo_t": wo_t,
            "bq2": bq2, "bk2": bk2, "bv_b": bv_b, "bo_b": bo_b,
            "sel_in": sel_np,
        })

    if _PROG is None:
        _PROG = _build()

    res = run_bass_kernel_spmd(_PROG, in_maps, core_ids=list(range(NCORES)),
                               trace=TRACE)
    LAST_EXEC_NS = res.exec_time_ns
    LAST_RESULTS = res

    out_full = np.empty((B, S, D), dtype=f32)
    for r in range(NCORES):
        b, c = divmod(r, GPB)
        q0 = QB * c
        out_full[b, q0:q0 + QB, :] = res.results[r]["out"]
    return out_full
